# revision 4
# baseline (speedup 1.0000x reference)
import sys
sys.path.insert(0, '/opt/trn_rl_repo')

"""Multi-head attention TP kernel for TRN2 — per-core program builder.

Sharding: 8 cores = 2 (batch) x 4 (head groups of 4 heads = 512 dims).
Each core computes, for its batch b and head-dim slice e:
    q = x[b] @ wq[e,:].T + bq[e]      (stored transposed: qT [E, S])
    k = x[b] @ wk[e,:].T + bk[e]      (kT [E, S])
    v = x[b] @ wv[e,:].T + bv[e]      (v [S, E])
    per head h (dh=128): ST = K-major score tiles, exp (no max-sub; scores
    bounded ~|3|), softmax denominator via DVE tensor_add accumulation of
    exp tiles + one all-ones cross-partition matmul, AV accumulated
    unnormalized, normalized on eviction.
    partial_out = attn_out @ wo[:, e].T   ([S, D]; host sums 8 partials + bo)

v2: one software-pipelined instruction stream, tuned from perfetto traces:
- The scalar-engine exp (~0.7-0.8 us per [128,512] tile, ~180 us/core
  total) hides under tensor-engine work: next-head K/Q projections and WO
  chunks are interleaved as "fillers" into the attention j-loops via a
  closure queue (2 pops/j, 5 during the last head when WO is available).
- AV matmuls are emitted 2 j-slots late (AV14/AV15 spill into the next
  block) so the last exp of a block never head-of-line-blocks the
  in-order PE queue; block finalize is deferred to the next block's j==1.
- Softmax denominator: DVE bf16 tensor_add chain over exp tiles + one
  all-ones cross-partition matmul; 1/denom via reciprocal_approx_fast
  (the exact DVE reciprocal costs 3.4 us/tile); V bias applied after
  normalization (softmax rows sum to 1), saving the bias matmuls.
- Q/K/V evictions on DVE, WO evictions on ACT (DVE congestion at the
  WO-heavy tail otherwise stalls the PE via the normalize chain).
- Startup: K/Q(h0) stream k-tile-wise under the x DMA (packed head-0
  weight columns arrive first; each dma_start costs ~0.74 us of serial
  Sync-engine trigger, so call count is minimized), with 3 V it-groups
  folded in; PSUM banks: 4 K + 1 Q + 3 V at startup, then 2 scores +
  2 AV-accum + 1 denom + 3 filler groups in steady state.
Measured ~363-366 us on 8 trn2 cores (vs 605 us for the 4-phase v1).
"""

import math
from collections import deque

import numpy as np

import concourse.bass as bass
import concourse.tile as tile
from concourse import bacc, mybir

F32 = mybir.dt.float32
BF16 = mybir.dt.bfloat16
AF = mybir.ActivationFunctionType


def build_module(
    S=2048,          # sequence per core (one batch)
    D=2048,          # model dim
    E=512,           # head dims per core (4 heads x 128)
    enable_asserts=False,
):
    HD = 128
    SC = 512
    NK = D // HD        # proj contraction tiles
    NH = E // HD        # heads per core
    NSC = S // SC       # s-chunks / i-blocks
    NJ = S // HD        # attention j tiles
    ND = D // SC        # WO n-chunks
    scale = 1.0 / math.sqrt(HD)

    nc = bacc.Bacc(
        "TRN2",
        target_bir_lowering=False,
        debug=False,
        enable_asserts=enable_asserts,
        num_devices=8,
    )

    xr = nc.dram_tensor("xr", [HD, NK * S], BF16, kind="ExternalInput").ap()
    wqt = nc.dram_tensor("wqt", [HD, NK * E], BF16, kind="ExternalInput").ap()
    wkt = nc.dram_tensor("wkt", [HD, NK * E], BF16, kind="ExternalInput").ap()
    # head-0 columns packed contiguously for a fast startup DMA
    wqh = nc.dram_tensor("wqh", [HD, NK * HD], BF16, kind="ExternalInput").ap()
    wkh = nc.dram_tensor("wkh", [HD, NK * HD], BF16, kind="ExternalInput").ap()
    wvt = nc.dram_tensor("wvt", [HD, NK * E], BF16, kind="ExternalInput").ap()
    wot = nc.dram_tensor("wot", [HD, NH * D], BF16, kind="ExternalInput").ap()
    bqc = nc.dram_tensor("bqc", [HD, NH], F32, kind="ExternalInput").ap()
    bkc = nc.dram_tensor("bkc", [HD, NH], F32, kind="ExternalInput").ap()
    bvc = nc.dram_tensor("bvc", [HD, NH], F32, kind="ExternalInput").ap()
    ones2d = nc.dram_tensor("ones2d", [HD, HD], BF16,
                            kind="ExternalInput").ap()
    out = nc.dram_tensor("out", [S, D], BF16, kind="ExternalOutput").ap()

    with tile.TileContext(nc) as tc:
        with (
            tc.tile_pool(name="main", bufs=1) as main,
            tc.tile_pool(name="consts", bufs=1) as consts,
        ):
            xall = main.tile([HD, NK, S], BF16)
            wq_sb = main.tile([HD, NK, E], BF16)
            wk_sb = main.tile([HD, NK, E], BF16)
            wqh_sb = main.tile([HD, NK, HD], BF16)
            wkh_sb = main.tile([HD, NK, HD], BF16)
            wv_sb = main.tile([HD, NK, E], BF16)
            wo_sb = main.tile([HD, NH, D], BF16)
            k_sb = main.tile([HD, 2, S], BF16)      # head ping-pong, kT
            v_sb = main.tile([HD, NJ, E], BF16)     # all heads
            q_sb = main.tile([HD, 2, SC], BF16)     # (h, ib) ping-pong
            outT = main.tile([HD, NH, S], BF16)

            bq_sb = consts.tile([HD, NH], F32)
            bk_sb = consts.tile([HD, NH], F32)
            bv_sb = consts.tile([HD, NH], F32)
            allones = consts.tile([HD, HD], BF16)

            # Two hardware DGE queues: Sync streams x exclusively (the
            # startup k-stream consumes one x k-tile per ~1.7 us, so x must
            # never queue behind weight chunks), Scalar/Activation carries
            # every weight + const (idle until the first exp at ~95 us).
            # Each dma_start costs a ~0.6-0.7 us serial trigger on its
            # engine, so call count still matters per queue.
            NVS = 3   # V it-groups folded into the startup k-stream
            HK = NK // 2

            def dma_wv(g4):
                nc.scalar.dma_start(
                    out=wv_sb[:, 4 * g4:4 * g4 + 4, :],
                    in_=wvt[:, 4 * g4 * E:(4 * g4 + 4) * E].rearrange(
                        "p (k e) -> p k e", e=E))

            # Sync queue: x only, finest chunks first.
            nc.sync.dma_start(out=xall[:, 0, 0:SC], in_=xr[:, 0:SC])
            nc.sync.dma_start(out=xall[:, 0, SC:], in_=xr[:, SC:S])
            nc.sync.dma_start(out=xall[:, 1, :], in_=xr[:, S:2 * S])
            for g in range(1, 8):
                nc.sync.dma_start(
                    out=xall[:, 2 * g:2 * g + 2, :],
                    in_=xr[:, 2 * g * S:(2 * g + 2) * S].rearrange(
                        "p (k s) -> p k s", s=S))

            # Scalar queue: startup weights first (wkh/wqh k0-half feeds the
            # first matmul), then wv for the folded V groups, biases before
            # the first K eviction, steady-state weights last.
            nc.scalar.dma_start(
                out=wkh_sb[:, 0:HK, :],
                in_=wkh[:, :HK * HD].rearrange("p (k e) -> p k e", e=HD))
            nc.scalar.dma_start(
                out=wqh_sb[:, 0:HK, :],
                in_=wqh[:, :HK * HD].rearrange("p (k e) -> p k e", e=HD))
            nc.scalar.dma_start(
                out=wkh_sb[:, HK:, :],
                in_=wkh[:, HK * HD:].rearrange("p (k e) -> p k e", e=HD))
            nc.scalar.dma_start(
                out=wqh_sb[:, HK:, :],
                in_=wqh[:, HK * HD:].rearrange("p (k e) -> p k e", e=HD))
            dma_wv(0)
            dma_wv(1)
            nc.scalar.dma_start(out=bk_sb, in_=bkc)
            dma_wv(2)
            dma_wv(3)
            nc.scalar.dma_start(out=bq_sb, in_=bqc)
            nc.scalar.dma_start(out=bv_sb, in_=bvc)
            nc.scalar.dma_start(out=allones, in_=ones2d)
            for g4 in range(NK // 4):
                nc.scalar.dma_start(
                    out=wk_sb[:, 4 * g4:4 * g4 + 4, :],
                    in_=wkt[:, 4 * g4 * E:(4 * g4 + 4) * E].rearrange(
                        "p (k e) -> p k e", e=E))
            for g4 in range(NK // 4):
                nc.scalar.dma_start(
                    out=wq_sb[:, 4 * g4:4 * g4 + 4, :],
                    in_=wqt[:, 4 * g4 * E:(4 * g4 + 4) * E].rearrange(
                        "p (k e) -> p k e", e=E))
            nc.scalar.dma_start(
                out=wo_sb, in_=wot.rearrange("p (k d) -> p k d", d=D))

            # ---- startup: K(h0), Q(h0, ib0), V it 0..NVS-1 k-tile-wise,
            # ---- then the remaining V it-groups.
            with tc.tile_pool(name="psStart", bufs=1, space="PSUM") as psS0:
                psK0 = [psS0.tile([HD, SC], F32, tag=f"k{si}",
                                  name=f"psk{si}") for si in range(NSC)]
                psQ0 = psS0.tile([HD, SC], F32, tag="q", name="psq0")
                psV0 = [psS0.tile([HD, E], F32, tag=f"v{it}",
                                  name=f"psv{it}") for it in range(NVS)]

                # PE warmup: the tensor engine clock ramps over ~3 us of
                # continuous work. Burn dummy matmuls on a memset scratch
                # tile during the otherwise-idle DMA-latency window so the
                # first real matmuls run at full clock.
                N_WARM, WARM_COLS = 20, 256
                warm = consts.tile([HD, WARM_COLS], BF16)
                nc.gpsimd.memset(warm, 0)
                for _ in range(N_WARM):
                    nc.tensor.matmul(psK0[0][:, 0:WARM_COLS],
                                     warm[:, 0:HD], warm,
                                     start=True, stop=True)
                # V trails K/Q by LAG k-tiles so its wv chunks (DMA'd
                # behind x) always arrive before the in-order PE stream
                # reaches them
                LAG = 4
                for kk in range(NK + LAG):
                    if kk < NK:
                        st, sp = kk == 0, kk == NK - 1
                        for si in range(NSC):
                            nc.tensor.matmul(
                                psK0[si], wkh_sb[:, kk, :],
                                xall[:, kk, si * SC:(si + 1) * SC],
                                start=st, stop=sp)
                        nc.tensor.matmul(
                            psQ0, wqh_sb[:, kk, :], xall[:, kk, 0:SC],
                            start=st, stop=sp)
                    vk = kk - LAG
                    if 0 <= vk < NK:
                        for it in range(NVS):
                            nc.tensor.matmul(
                                psV0[it],
                                xall[:, vk, it * HD:(it + 1) * HD],
                                wv_sb[:, vk, :], start=(vk == 0),
                                stop=(vk == NK - 1))
                for si in range(NSC):
                    nc.vector.tensor_scalar_add(
                        k_sb[:, 0, si * SC:(si + 1) * SC], psK0[si],
                        bk_sb[:, 0:1])
                nc.vector.tensor_scalar_add(q_sb[:, 0, :], psQ0, bq_sb[:, 0:1])
                for it in range(NVS):
                    nc.vector.tensor_copy(v_sb[:, it, :], psV0[it])

                for it in range(NVS, NJ):
                    psV = psS0.tile([HD, E], F32, tag=f"v{it % NVS}",
                                    name="psv")
                    for kk in range(NK):
                        nc.tensor.matmul(
                            psV, xall[:, kk, it * HD:(it + 1) * HD],
                            wv_sb[:, kk, :], start=(kk == 0),
                            stop=(kk == NK - 1))
                    nc.vector.tensor_copy(v_sb[:, it, :], psV)

            # ---------------- pipelined attention + fillers ----------------
            with (
                tc.tile_pool(name="psS", bufs=2, space="PSUM") as psS_pool,
                tc.tile_pool(name="psO", bufs=2, space="PSUM") as psO_pool,
                tc.tile_pool(name="psBC", bufs=1, space="PSUM") as psBC_pool,
                tc.tile_pool(name="psF", bufs=3, space="PSUM") as psF_pool,
                tc.tile_pool(name="es", bufs=6) as es_pool,
                tc.tile_pool(name="accp", bufs=3) as acc_pool,
                tc.tile_pool(name="recipp", bufs=2) as recip_pool,
                tc.tile_pool(name="og", bufs=2) as og_pool,
            ):
                filler = deque()

                def emit_filler(n):
                    for _ in range(n):
                        if filler:
                            filler.popleft()()

                def enq_q(h, ib, qb):
                    psQ = psF_pool.tile([HD, SC], F32, tag="f", name="psq")

                    def op(kk, psQ=psQ, h=h, ib=ib, qb=qb):
                        nc.tensor.matmul(
                            psQ, wq_sb[:, kk, h * HD:(h + 1) * HD],
                            xall[:, kk, ib * SC:(ib + 1) * SC],
                            start=(kk == 0), stop=(kk == NK - 1))
                        if kk == NK - 1:
                            nc.vector.tensor_scalar_add(
                                q_sb[:, qb, :], psQ, bq_sb[:, h:h + 1])
                    for kk in range(NK):
                        filler.append(lambda kk=kk, op=op: op(kk))

                def enq_k(h, si, kb):
                    psK = psF_pool.tile([HD, SC], F32, tag="f", name="psk")

                    def op(kk, psK=psK, h=h, si=si, kb=kb):
                        nc.tensor.matmul(
                            psK, wk_sb[:, kk, h * HD:(h + 1) * HD],
                            xall[:, kk, si * SC:(si + 1) * SC],
                            start=(kk == 0), stop=(kk == NK - 1))
                        if kk == NK - 1:
                            nc.vector.tensor_scalar_add(
                                k_sb[:, kb, si * SC:(si + 1) * SC], psK,
                                bk_sb[:, h:h + 1])
                    for kk in range(NK):
                        filler.append(lambda kk=kk, op=op: op(kk))

                def enq_wo(ibp):
                    for it4 in range(4):
                        it = ibp * 4 + it4
                        og = og_pool.tile([HD, D], BF16, tag="og", name="og")
                        for nn in range(ND):
                            psW = psF_pool.tile([HD, SC], F32, tag="f",
                                                name="psw")

                            def op(kkh, psW=psW, og=og, it=it, nn=nn, ibp=ibp):
                                nc.tensor.matmul(
                                    psW,
                                    outT[:, kkh, it * HD:(it + 1) * HD],
                                    wo_sb[:, kkh, nn * SC:(nn + 1) * SC],
                                    start=(kkh == 0), stop=(kkh == NH - 1))
                                if kkh == NH - 1:
                                    if ibp == 3:
                                        # drain: both ACT and DVE are idle;
                                        # alternate so the eviction chain
                                        # keeps pace with the WO matmuls
                                        if (it * ND + nn) % 2:
                                            nc.scalar.copy(
                                                og[:, nn * SC:(nn + 1) * SC],
                                                psW)
                                        else:
                                            nc.vector.tensor_copy(
                                                og[:, nn * SC:(nn + 1) * SC],
                                                psW)
                                    elif ibp == 2:
                                        # last attention block: DVE (ACT
                                        # paces the exp chain there)
                                        nc.vector.tensor_copy(
                                            og[:, nn * SC:(nn + 1) * SC], psW)
                                    else:
                                        nc.scalar.copy(
                                            og[:, nn * SC:(nn + 1) * SC], psW)
                                    if it == NJ - 1:
                                        # tail: per-chunk DMA right after
                                        # each eviction so the final
                                        # transfer overlaps the drain
                                        nc.sync.dma_start(
                                            out=out[it * HD:(it + 1) * HD,
                                                    nn * SC:(nn + 1) * SC],
                                            in_=og[:, nn * SC:(nn + 1) * SC])
                                    elif nn == ND - 1:
                                        nc.sync.dma_start(
                                            out=out[it * HD:(it + 1) * HD, :],
                                            in_=og)
                            for kkh in range(NH):
                                filler.append(lambda kkh=kkh, op=op: op(kkh))

                pending_fin = [None]
                # AV matmuls are emitted 2 j-slots late: the last exp of a
                # block lags the PE by ~1.4 us, and an eagerly-emitted AV15
                # head-of-line-blocks the in-order PE queue at every block
                # boundary. AV14/AV15 spill into the next block's stream.
                av_q = deque()

                for h in range(NH):
                    kb = h % 2
                    for ib in range(NSC):
                        i0 = ib * SC
                        qb = (h * NSC + ib) % 2
                        if ib < NSC - 1:
                            enq_q(h, ib + 1, qb ^ 1)
                        elif h < NH - 1:
                            enq_q(h + 1, 0, qb ^ 1)
                        if h < NH - 1:
                            enq_k(h + 1, ib, kb ^ 1)

                        if h == NH - 1:
                            # leave ~16 WO fillers for the epilogue: the
                            # deferred AV14/AV15 flush needs covering work
                            # or it stalls on the final block's last exp
                            npop = 4 if ib == NSC - 1 else 5
                        else:
                            npop = 2
                        psO = psO_pool.tile([HD, SC], F32, tag="o", name="pso")
                        es_t = [None] * NJ
                        acc = None

                        def emit_av(j, h=h, psO=psO, es_t=es_t):
                            nc.tensor.matmul(
                                psO, v_sb[:, j, h * HD:(h + 1) * HD],
                                es_t[j],
                                start=(j == 0), stop=(j == NJ - 1))

                        for j in range(NJ):
                            psS = psS_pool.tile([HD, SC], F32, tag="s",
                                                name="pss")
                            nc.tensor.matmul(
                                psS, k_sb[:, kb, j * HD:(j + 1) * HD],
                                q_sb[:, qb, :], start=True, stop=True)
                            emit_filler(npop)
                            if len(av_q) >= 2:
                                av_q.popleft()()
                            if j == 1:
                                # after the AV15 pop above — fin's normalize
                                # reads psO and must be emitted after it
                                if pending_fin[0] is not None:
                                    pending_fin[0]()
                                    pending_fin[0] = None
                                if h == NH - 1 and ib >= 1:
                                    enq_wo(ib - 1)
                            es = es_pool.tile([HD, SC], BF16, tag="es",
                                              name="es")
                            nc.scalar.activation(es, psS, AF.Exp, scale=scale)
                            es_t[j] = es
                            av_q.append(lambda j=j, f=emit_av: f(j))
                            if j >= 1:
                                nacc = acc_pool.tile([HD, SC], BF16, tag="acc",
                                                     name="acc")
                                if j == 1:
                                    nc.vector.tensor_add(nacc, es_t[0], es)
                                else:
                                    nc.vector.tensor_add(nacc, acc, es)
                                acc = nacc

                        def fin(h=h, i0=i0, psO=psO, acc=acc):
                            psBC = psBC_pool.tile([HD, SC], F32, tag="bc",
                                                  name="psbc")
                            nc.tensor.matmul(psBC, allones, acc,
                                             start=True, stop=True)
                            recip = recip_pool.tile([HD, SC], F32, tag="r",
                                                    name="recip")
                            nc.vector.reciprocal_approx_fast(
                                out=recip, in_=psBC)
                            nc.vector.tensor_mul(
                                outT[:, h, i0:i0 + SC], psO, recip)
                            nc.vector.tensor_scalar_add(
                                outT[:, h, i0:i0 + SC],
                                outT[:, h, i0:i0 + SC], bv_sb[:, h:h + 1])
                        pending_fin[0] = fin

                # epilogue: cover the exp15/normalize latency of the last
                # block with leftover WO fillers before flushing the
                # deferred AVs and the final WO chunk
                emit_filler(min(len(filler), 8))
                while av_q:
                    av_q.popleft()()
                pending_fin[0]()
                pending_fin[0] = None
                emit_filler(min(len(filler), 12))
                enq_wo(NSC - 1)
                emit_filler(len(filler))

    nc.compile()
    return nc


# ---------------------------------------------------------------------------
# Host-side sharding helpers
# ---------------------------------------------------------------------------

def _bf16(a):
    import ml_dtypes
    return np.asarray(a).astype(ml_dtypes.bfloat16)


def make_in_map(x_b, wq_e, bq_e, wk_e, bk_e, wv_e, bv_e, wo_e):
    """Per-core input dict. x_b [S, D]; w*_e [E, D] row slices; wo_e [D, E]
    column slice; b*_e [E]."""
    E = wq_e.shape[0]
    S, D = x_b.shape
    HD = 128
    NH = E // HD
    NK = D // HD

    def wrelayout(wT):  # [D, E'] -> [HD, NK*E'] with k-tile-major columns
        Ew = wT.shape[1]
        return _bf16(
            wT.reshape(NK, HD, Ew).transpose(1, 0, 2).reshape(HD, NK * Ew))

    xT = x_b.T  # [D, S]
    return {
        "xr": _bf16(xT.reshape(NK, HD, S).transpose(1, 0, 2)
                    .reshape(HD, NK * S)),
        "wqt": wrelayout(wq_e.T),
        "wkt": wrelayout(wk_e.T),
        "wqh": wrelayout(np.ascontiguousarray(wq_e.T[:, 0:HD])),
        "wkh": wrelayout(np.ascontiguousarray(wk_e.T[:, 0:HD])),
        "wvt": wrelayout(wv_e.T),
        "wot": _bf16(
            wo_e.T.reshape(NH, HD, D).transpose(1, 0, 2).reshape(HD, NH * D)),
        "bqc": np.ascontiguousarray(bq_e.reshape(NH, HD).T),
        "bkc": np.ascontiguousarray(bk_e.reshape(NH, HD).T),
        "bvc": np.ascontiguousarray(bv_e.reshape(NH, HD).T),
        "ones2d": _bf16(np.ones((HD, HD), np.float32)),
    }


# ---------------------------------------------------------------------------
# Entry point: full-input kernel with internal 8-way sharding
# ---------------------------------------------------------------------------

import os as _os

_NC_CACHE = {}


def _get_module():
    if "nc" not in _NC_CACHE:
        _NC_CACHE["nc"] = build_module(S=2048, D=2048, E=512)
    return _NC_CACHE["nc"]


def kernel(x, wq, bq, wk, bk, wv, bv, wo, bo):
    """Full inputs -> full output. 8 cores = 2 (batch) x 4 (head-group)."""
    from concourse import bass_utils

    x = np.asarray(x, dtype=np.float32)
    wq, bq = np.asarray(wq, np.float32), np.asarray(bq, np.float32)
    wk, bk = np.asarray(wk, np.float32), np.asarray(bk, np.float32)
    wv, bv = np.asarray(wv, np.float32), np.asarray(bv, np.float32)
    wo, bo = np.asarray(wo, np.float32), np.asarray(bo, np.float32)

    E = 512
    nc = _get_module()
    in_maps = []
    for c in range(8):
        b, g = divmod(c, 4)
        e = slice(g * E, (g + 1) * E)
        in_maps.append(make_in_map(
            x[b], wq[e], bq[e], wk[e], bk[e], wv[e], bv[e], wo[:, e]))

    trace = bool(int(_os.environ.get("ATTN_TRACE", "0")))
    kw = {}
    if trace:
        tmpdir = _os.environ.get("ATTN_TRACE_DIR") or None
        kw = dict(trace=True, tmpdir=tmpdir, trace_cores=[0])
    res = bass_utils.run_bass_kernel_spmd(
        nc, in_maps, core_ids=list(range(8)), **kw)
    if trace:
        print(f"HW exec time: {res.exec_time_ns} ns")
        _NC_CACHE["last_results"] = res

    y = np.empty((2, 2048, 2048), np.float32)
    for b in range(2):
        acc = np.asarray(res.results[4 * b]["out"], dtype=np.float32)
        for g in range(1, 4):
            acc += np.asarray(res.results[4 * b + g]["out"], dtype=np.float32)
        y[b] = acc + bo
    return y



# revision 16
# speedup vs baseline: 1.0060x; 1.0060x over previous
import sys
sys.path.insert(0, '/opt/trn_rl_repo')

"""Multi-head attention TP kernel for TRN2 — per-core program builder.

Sharding: 8 cores = 2 (batch) x 4 (head groups of 4 heads = 512 dims).
Each core computes, for its batch b and head-dim slice e:
    q = x[b] @ wq[e,:].T + bq[e]      (stored transposed: qT [E, S])
    k = x[b] @ wk[e,:].T + bk[e]      (kT [E, S])
    v = x[b] @ wv[e,:].T + bv[e]      (v [S, E])
    per head h (dh=128): ST = K-major score tiles, exp (no max-sub; scores
    bounded ~|3|), softmax denominator via DVE tensor_add accumulation of
    exp tiles + one all-ones cross-partition matmul, AV accumulated
    unnormalized, normalized on eviction.
    partial_out = attn_out @ wo[:, e].T   ([S, D]; host sums 8 partials + bo)

v2: one software-pipelined instruction stream, tuned from perfetto traces:
- The scalar-engine exp (~0.7-0.8 us per [128,512] tile, ~180 us/core
  total) hides under tensor-engine work: next-head K/Q projections and WO
  chunks are interleaved as "fillers" into the attention j-loops via a
  closure queue (2 pops/j, 5 during the last head when WO is available).
- AV matmuls are emitted 2 j-slots late (AV14/AV15 spill into the next
  block) so the last exp of a block never head-of-line-blocks the
  in-order PE queue; block finalize is deferred to the next block's j==1.
- Softmax denominator: DVE bf16 tensor_add chain over exp tiles + one
  all-ones cross-partition matmul; 1/denom via reciprocal_approx_fast
  (the exact DVE reciprocal costs 3.4 us/tile); V bias applied after
  normalization (softmax rows sum to 1), saving the bias matmuls.
- Q/K/V evictions on DVE, WO evictions on ACT (DVE congestion at the
  WO-heavy tail otherwise stalls the PE via the normalize chain).
- Startup: K/Q(h0) stream k-tile-wise under the x DMA (packed head-0
  weight columns arrive first; each dma_start costs ~0.74 us of serial
  Sync-engine trigger, so call count is minimized), with 3 V it-groups
  folded in; PSUM banks: 4 K + 1 Q + 3 V at startup, then 2 scores +
  2 AV-accum + 1 denom + 3 filler groups in steady state.
Measured ~363-366 us on 8 trn2 cores (vs 605 us for the 4-phase v1).
"""

import math
from collections import deque

import numpy as np

import concourse.bass as bass
import concourse.tile as tile
from concourse import bacc, mybir

F32 = mybir.dt.float32
BF16 = mybir.dt.bfloat16
AF = mybir.ActivationFunctionType


def build_module(
    S=2048,          # sequence per core (one batch)
    D=2048,          # model dim
    E=512,           # head dims per core (4 heads x 128)
    enable_asserts=False,
):
    HD = 128
    SC = 512
    NK = D // HD        # proj contraction tiles
    NH = E // HD        # heads per core
    NSC = S // SC       # s-chunks / i-blocks
    NJ = S // HD        # attention j tiles
    ND = D // SC        # WO n-chunks
    scale = 1.0 / math.sqrt(HD)

    # fp8 tensor-engine path for the steady-state K/Q projection fillers
    # (heads 1-3): DoubleRow perf mode processes two 128-row k-tiles per
    # matmul at 2x bf16 throughput. Weights are host-prescaled by 2**13 to
    # lift them out of e4m3's denormal range; the factor is divided back
    # out inside the exp activation's scale argument (biases are
    # host-prescaled to match), so descaling costs zero instructions.
    # Head 0 stays bf16: its K/Q run in the DMA-bound startup stream where
    # PE savings buy nothing, and keeping it unscaled lets bias column 0
    # serve the bf16 path unchanged.
    W8SCALE = 2.0 ** 13
    F8 = mybir.dt.float8e4

    nc = bacc.Bacc(
        "TRN2",
        target_bir_lowering=False,
        debug=False,
        enable_asserts=enable_asserts,
        num_devices=8,
    )

    xr = nc.dram_tensor("xr", [HD, NK * S], BF16, kind="ExternalInput").ap()
    wqt = nc.dram_tensor("wqt", [HD, NK * E], F8, kind="ExternalInput").ap()
    wkt = nc.dram_tensor("wkt", [HD, NK * E], F8, kind="ExternalInput").ap()
    # head-0 columns packed contiguously for a fast startup DMA
    wqh = nc.dram_tensor("wqh", [HD, NK * HD], BF16, kind="ExternalInput").ap()
    wkh = nc.dram_tensor("wkh", [HD, NK * HD], BF16, kind="ExternalInput").ap()
    wvt = nc.dram_tensor("wvt", [HD, NK * E], BF16, kind="ExternalInput").ap()
    wot = nc.dram_tensor("wot", [HD, NH * D], BF16, kind="ExternalInput").ap()
    bqc = nc.dram_tensor("bqc", [HD, NH], F32, kind="ExternalInput").ap()
    bkc = nc.dram_tensor("bkc", [HD, NH], F32, kind="ExternalInput").ap()
    bvc = nc.dram_tensor("bvc", [HD, NH], F32, kind="ExternalInput").ap()
    ones2d = nc.dram_tensor("ones2d", [HD, HD], BF16,
                            kind="ExternalInput").ap()
    out = nc.dram_tensor("out", [S, D], BF16, kind="ExternalOutput").ap()

    with tile.TileContext(nc) as tc:
        with (
            tc.tile_pool(name="main", bufs=1) as main,
            tc.tile_pool(name="consts", bufs=1) as consts,
        ):
            xall = main.tile([HD, NK, S], BF16)
            x8 = main.tile([HD, NK, S], F8)     # gpsimd-cast copy of x
            wq_sb = main.tile([HD, NK, E], F8)
            wk_sb = main.tile([HD, NK, E], F8)
            wqh_sb = main.tile([HD, NK, HD], BF16)
            wo_sb = main.tile([HD, NH, D], BF16)
            k_sb = main.tile([HD, 2, S], BF16)      # head ping-pong, kT
            v_sb = main.tile([HD, NJ, E], BF16)     # all heads
            q_sb = main.tile([HD, 2, SC], BF16)     # (h, ib) ping-pong
            outT = main.tile([HD, NH, S], BF16)

            bq_sb = consts.tile([HD, NH], F32)
            bk_sb = consts.tile([HD, NH], F32)
            bv_sb = consts.tile([HD, NH], F32)
            allones = consts.tile([HD, HD], BF16)

            # Startup-only tiles live in a scoped pool: closing it after the
            # startup phase releases their SBUF to the attention-phase pools
            # (the allocator tracks released-zone overlap dependencies).
            su_pool = tc.tile_pool(name="su", bufs=1)
            su = su_pool.__enter__()
            wkh_sb = su.tile([HD, NK, HD], BF16)
            wv_sb = su.tile([HD, NK, E], BF16)

            # Two hardware DGE queues: Sync streams x exclusively (the
            # startup k-stream consumes one x k-tile per ~1.7 us, so x must
            # never queue behind weight chunks), Scalar/Activation carries
            # every weight + const (idle until the first exp at ~95 us).
            # Each dma_start costs a ~0.6-0.7 us serial trigger on its
            # engine, so call count still matters per queue.
            NVS = 3   # V it-groups folded into the startup k-stream
            HK = NK // 2

            def dma_wv(g4):
                nc.scalar.dma_start(
                    out=wv_sb[:, 4 * g4:4 * g4 + 4, :],
                    in_=wvt[:, 4 * g4 * E:(4 * g4 + 4) * E].rearrange(
                        "p (k e) -> p k e", e=E))

            # Sync queue: x only, finest chunks first.
            nc.sync.dma_start(out=xall[:, 0, 0:SC], in_=xr[:, 0:SC])
            nc.sync.dma_start(out=xall[:, 0, SC:], in_=xr[:, SC:S])
            nc.sync.dma_start(out=xall[:, 1, :], in_=xr[:, S:2 * S])
            for g in range(1, 8):
                nc.sync.dma_start(
                    out=xall[:, 2 * g:2 * g + 2, :],
                    in_=xr[:, 2 * g * S:(2 * g + 2) * S].rearrange(
                        "p (k s) -> p k s", s=S))

            # Scalar queue: startup weights first (wkh/wqh k0-half feeds the
            # first matmul), then wv for the folded V groups, biases before
            # the first K eviction, steady-state weights last.
            nc.scalar.dma_start(
                out=wkh_sb[:, 0:HK, :],
                in_=wkh[:, :HK * HD].rearrange("p (k e) -> p k e", e=HD))
            nc.scalar.dma_start(
                out=wqh_sb[:, 0:HK, :],
                in_=wqh[:, :HK * HD].rearrange("p (k e) -> p k e", e=HD))
            nc.scalar.dma_start(
                out=wkh_sb[:, HK:, :],
                in_=wkh[:, HK * HD:].rearrange("p (k e) -> p k e", e=HD))
            nc.scalar.dma_start(
                out=wqh_sb[:, HK:, :],
                in_=wqh[:, HK * HD:].rearrange("p (k e) -> p k e", e=HD))
            dma_wv(0)
            dma_wv(1)
            nc.scalar.dma_start(out=bk_sb, in_=bkc)
            dma_wv(2)
            dma_wv(3)
            nc.scalar.dma_start(out=bq_sb, in_=bqc)
            nc.scalar.dma_start(out=bv_sb, in_=bvc)
            nc.scalar.dma_start(out=allones, in_=ones2d)
            for g4 in range(NK // 4):
                nc.scalar.dma_start(
                    out=wk_sb[:, 4 * g4:4 * g4 + 4, :],
                    in_=wkt[:, 4 * g4 * E:(4 * g4 + 4) * E].rearrange(
                        "p (k e) -> p k e", e=E))
            for g4 in range(NK // 4):
                nc.scalar.dma_start(
                    out=wq_sb[:, 4 * g4:4 * g4 + 4, :],
                    in_=wqt[:, 4 * g4 * E:(4 * g4 + 4) * E].rearrange(
                        "p (k e) -> p k e", e=E))
            nc.scalar.dma_start(
                out=wo_sb, in_=wot.rearrange("p (k d) -> p k d", d=D))

            # x -> fp8 casts on the otherwise-idle gpsimd engine, k-tile by
            # k-tile as the x DMAs land. Consumed by the h1-3 K/Q fillers,
            # whose first pop is ~60us after the last cast completes.
            for kk in range(NK):
                for c in range(NSC):
                    nc.gpsimd.tensor_copy(
                        x8[:, kk, c * SC:(c + 1) * SC],
                        xall[:, kk, c * SC:(c + 1) * SC])

            # ---- startup: K(h0), Q(h0, ib0), V it 0..NVS-1 k-tile-wise,
            # ---- then the remaining V it-groups.
            with tc.tile_pool(name="psStart", bufs=1, space="PSUM") as psS0:
                psK0 = [psS0.tile([HD, SC], F32, tag=f"k{si}",
                                  name=f"psk{si}") for si in range(NSC)]
                psQ0 = psS0.tile([HD, SC], F32, tag="q", name="psq0")
                psV0 = [psS0.tile([HD, E], F32, tag=f"v{it}",
                                  name=f"psv{it}") for it in range(NVS)]

                # PE warmup: the tensor engine clock ramps over ~3 us of
                # continuous work. Burn dummy matmuls on a memset scratch
                # tile during the otherwise-idle DMA-latency window so the
                # first real matmuls run at full clock.
                N_WARM, WARM_COLS = 20, 256
                warm = su.tile([HD, WARM_COLS], BF16)
                nc.gpsimd.memset(warm, 0)
                for _ in range(N_WARM):
                    nc.tensor.matmul(psK0[0][:, 0:WARM_COLS],
                                     warm[:, 0:HD], warm,
                                     start=True, stop=True)
                # V trails K/Q by LAG k-tiles so its wv chunks (DMA'd
                # behind x) always arrive before the in-order PE stream
                # reaches them
                LAG = 4
                for kk in range(NK + LAG):
                    if kk < NK:
                        st, sp = kk == 0, kk == NK - 1
                        for si in range(NSC):
                            nc.tensor.matmul(
                                psK0[si], wkh_sb[:, kk, :],
                                xall[:, kk, si * SC:(si + 1) * SC],
                                start=st, stop=sp)
                        nc.tensor.matmul(
                            psQ0, wqh_sb[:, kk, :], xall[:, kk, 0:SC],
                            start=st, stop=sp)
                    vk = kk - LAG
                    if 0 <= vk < NK:
                        for it in range(NVS):
                            nc.tensor.matmul(
                                psV0[it],
                                xall[:, vk, it * HD:(it + 1) * HD],
                                wv_sb[:, vk, :], start=(vk == 0),
                                stop=(vk == NK - 1))
                for si in range(NSC):
                    nc.vector.tensor_scalar_add(
                        k_sb[:, 0, si * SC:(si + 1) * SC], psK0[si],
                        bk_sb[:, 0:1])
                nc.vector.tensor_scalar_add(q_sb[:, 0, :], psQ0, bq_sb[:, 0:1])
                for it in range(NVS):
                    nc.vector.tensor_copy(v_sb[:, it, :], psV0[it])

                for it in range(NVS, NJ):
                    psV = psS0.tile([HD, E], F32, tag=f"v{it % NVS}",
                                    name="psv")
                    for kk in range(NK):
                        nc.tensor.matmul(
                            psV, xall[:, kk, it * HD:(it + 1) * HD],
                            wv_sb[:, kk, :], start=(kk == 0),
                            stop=(kk == NK - 1))
                    nc.vector.tensor_copy(v_sb[:, it, :], psV)

            su_pool.__exit__(None, None, None)

            # ---------------- pipelined attention + fillers ----------------
            with (
                tc.tile_pool(name="psS", bufs=2, space="PSUM") as psS_pool,
                tc.tile_pool(name="psO", bufs=2, space="PSUM") as psO_pool,
                tc.tile_pool(name="psBC", bufs=1, space="PSUM") as psBC_pool,
                tc.tile_pool(name="psF", bufs=3, space="PSUM") as psF_pool,
                tc.tile_pool(name="es", bufs=6) as es_pool,
                tc.tile_pool(name="accp", bufs=3) as acc_pool,
                tc.tile_pool(name="recipp", bufs=2) as recip_pool,
                tc.tile_pool(name="og", bufs=2) as og_pool,
            ):
                filler = deque()

                def emit_filler(n):
                    for _ in range(n):
                        if filler:
                            filler.popleft()()

                DR = mybir.MatmulPerfMode.DoubleRow
                NP = NK // 2    # DoubleRow k-tile pairs

                def enq_q(h, ib, qb):
                    psQ = psF_pool.tile([HD, SC], F32, tag="f", name="psq")
                    if h == 0:
                        # bf16 path (unscaled bias column 0)
                        def op(kk, psQ=psQ, ib=ib, qb=qb):
                            nc.tensor.matmul(
                                psQ, wqh_sb[:, kk, :],
                                xall[:, kk, ib * SC:(ib + 1) * SC],
                                start=(kk == 0), stop=(kk == NK - 1))
                            if kk == NK - 1:
                                nc.vector.tensor_scalar_add(
                                    q_sb[:, qb, :], psQ, bq_sb[:, 0:1])
                        for kk in range(NK):
                            filler.append(lambda kk=kk, op=op: op(kk))
                        return

                    def op(t, psQ=psQ, h=h, ib=ib, qb=qb):
                        nc.tensor.matmul(
                            psQ, wq_sb[:, 2 * t:2 * t + 2, h * HD:(h + 1) * HD],
                            x8[:, 2 * t:2 * t + 2, ib * SC:(ib + 1) * SC],
                            start=(t == 0), stop=(t == NP - 1), perf_mode=DR)
                        if t == NP - 1:
                            nc.vector.tensor_scalar_add(
                                q_sb[:, qb, :], psQ, bq_sb[:, h:h + 1])
                    for t in range(NP):
                        filler.append(lambda t=t, op=op: op(t))

                def enq_k(h, si, kb):
                    psK = psF_pool.tile([HD, SC], F32, tag="f", name="psk")

                    def op(t, psK=psK, h=h, si=si, kb=kb):
                        nc.tensor.matmul(
                            psK, wk_sb[:, 2 * t:2 * t + 2, h * HD:(h + 1) * HD],
                            x8[:, 2 * t:2 * t + 2, si * SC:(si + 1) * SC],
                            start=(t == 0), stop=(t == NP - 1), perf_mode=DR)
                        if t == NP - 1:
                            nc.vector.tensor_scalar_add(
                                k_sb[:, kb, si * SC:(si + 1) * SC], psK,
                                bk_sb[:, h:h + 1])
                    for t in range(NP):
                        filler.append(lambda t=t, op=op: op(t))

                def enq_wo(ibp):
                    for it4 in range(4):
                        it = ibp * 4 + it4
                        og = og_pool.tile([HD, D], BF16, tag="og", name="og")
                        for nn in range(ND):
                            psW = psF_pool.tile([HD, SC], F32, tag="f",
                                                name="psw")

                            def op(kkh, psW=psW, og=og, it=it, nn=nn, ibp=ibp):
                                nc.tensor.matmul(
                                    psW,
                                    outT[:, kkh, it * HD:(it + 1) * HD],
                                    wo_sb[:, kkh, nn * SC:(nn + 1) * SC],
                                    start=(kkh == 0), stop=(kkh == NH - 1))
                                if kkh == NH - 1:
                                    if ibp == 3:
                                        # drain: both ACT and DVE are idle;
                                        # alternate so the eviction chain
                                        # keeps pace with the WO matmuls
                                        if (it * ND + nn) % 2:
                                            nc.scalar.copy(
                                                og[:, nn * SC:(nn + 1) * SC],
                                                psW)
                                        else:
                                            nc.vector.tensor_copy(
                                                og[:, nn * SC:(nn + 1) * SC],
                                                psW)
                                    elif ibp == 2:
                                        # last attention block: DVE (ACT
                                        # paces the exp chain there)
                                        nc.vector.tensor_copy(
                                            og[:, nn * SC:(nn + 1) * SC], psW)
                                    else:
                                        nc.scalar.copy(
                                            og[:, nn * SC:(nn + 1) * SC], psW)
                                    if it == NJ - 1:
                                        # tail: per-chunk DMA right after
                                        # each eviction so the final
                                        # transfer overlaps the drain
                                        nc.sync.dma_start(
                                            out=out[it * HD:(it + 1) * HD,
                                                    nn * SC:(nn + 1) * SC],
                                            in_=og[:, nn * SC:(nn + 1) * SC])
                                    elif nn == ND - 1:
                                        nc.sync.dma_start(
                                            out=out[it * HD:(it + 1) * HD, :],
                                            in_=og)
                            for kkh in range(NH):
                                filler.append(lambda kkh=kkh, op=op: op(kkh))

                pending_fin = [None]
                # AV matmuls are emitted 2 j-slots late: the last exp of a
                # block lags the PE by ~1.4 us, and an eagerly-emitted AV15
                # head-of-line-blocks the in-order PE queue at every block
                # boundary. AV14/AV15 spill into the next block's stream.
                av_q = deque()

                for h in range(NH):
                    kb = h % 2
                    for ib in range(NSC):
                        i0 = ib * SC
                        qb = (h * NSC + ib) % 2
                        if ib < NSC - 1:
                            enq_q(h, ib + 1, qb ^ 1)
                        elif h < NH - 1:
                            enq_q(h + 1, 0, qb ^ 1)
                        if h < NH - 1:
                            enq_k(h + 1, ib, kb ^ 1)

                        if h == NH - 1:
                            # leave ~16 WO fillers for the epilogue: the
                            # deferred AV14/AV15 flush needs covering work
                            # or it stalls on the final block's last exp
                            npop = 4 if ib == NSC - 1 else 5
                        else:
                            npop = 2
                        psO = psO_pool.tile([HD, SC], F32, tag="o", name="pso")
                        es_t = [None] * NJ
                        acc = None

                        def emit_av(j, h=h, psO=psO, es_t=es_t):
                            nc.tensor.matmul(
                                psO, v_sb[:, j, h * HD:(h + 1) * HD],
                                es_t[j],
                                start=(j == 0), stop=(j == NJ - 1))

                        for j in range(NJ):
                            psS = psS_pool.tile([HD, SC], F32, tag="s",
                                                name="pss")
                            nc.tensor.matmul(
                                psS, k_sb[:, kb, j * HD:(j + 1) * HD],
                                q_sb[:, qb, :], start=True, stop=True)
                            emit_filler(npop)
                            if len(av_q) >= 2:
                                av_q.popleft()()
                            if j == 1:
                                # after the AV15 pop above — fin's normalize
                                # reads psO and must be emitted after it
                                if pending_fin[0] is not None:
                                    pending_fin[0]()
                                    pending_fin[0] = None
                                if h == NH - 1 and ib >= 1:
                                    enq_wo(ib - 1)
                            es = es_pool.tile([HD, SC], BF16, tag="es",
                                              name="es")
                            # h>=1 scores carry the 2**13 weight prescale on
                            # both q and k; divide it back out here for free
                            sc_h = scale if h == 0 else scale / (W8SCALE ** 2)
                            nc.scalar.activation(es, psS, AF.Exp, scale=sc_h)
                            es_t[j] = es
                            av_q.append(lambda j=j, f=emit_av: f(j))
                            if j >= 1:
                                nacc = acc_pool.tile([HD, SC], BF16, tag="acc",
                                                     name="acc")
                                if j == 1:
                                    nc.vector.tensor_add(nacc, es_t[0], es)
                                else:
                                    nc.vector.tensor_add(nacc, acc, es)
                                acc = nacc

                        def fin(h=h, i0=i0, psO=psO, acc=acc):
                            psBC = psBC_pool.tile([HD, SC], F32, tag="bc",
                                                  name="psbc")
                            nc.tensor.matmul(psBC, allones, acc,
                                             start=True, stop=True)
                            recip = recip_pool.tile([HD, SC], F32, tag="r",
                                                    name="recip")
                            nc.vector.reciprocal_approx_fast(
                                out=recip, in_=psBC)
                            nc.vector.tensor_mul(
                                outT[:, h, i0:i0 + SC], psO, recip)
                            nc.vector.tensor_scalar_add(
                                outT[:, h, i0:i0 + SC],
                                outT[:, h, i0:i0 + SC], bv_sb[:, h:h + 1])
                        pending_fin[0] = fin

                # epilogue: cover the exp15/normalize latency of the last
                # block with leftover WO fillers before flushing the
                # deferred AVs and the final WO chunk
                emit_filler(min(len(filler), 8))
                while av_q:
                    av_q.popleft()()
                pending_fin[0]()
                pending_fin[0] = None
                emit_filler(min(len(filler), 12))
                enq_wo(NSC - 1)
                emit_filler(len(filler))

    nc.compile()
    return nc


# ---------------------------------------------------------------------------
# Host-side sharding helpers
# ---------------------------------------------------------------------------

def _bf16(a):
    import ml_dtypes
    return np.asarray(a).astype(ml_dtypes.bfloat16)


def _f8(a):
    import ml_dtypes
    return np.asarray(a).astype(ml_dtypes.float8_e4m3)


def make_in_map(x_b, wq_e, bq_e, wk_e, bk_e, wv_e, bv_e, wo_e):
    """Per-core input dict. x_b [S, D]; w*_e [E, D] row slices; wo_e [D, E]
    column slice; b*_e [E]."""
    E = wq_e.shape[0]
    S, D = x_b.shape
    HD = 128
    NH = E // HD
    NK = D // HD

    W8SCALE = np.float32(2.0 ** 13)

    def relayout(wT):  # [D, E'] -> [HD, NK*E'] with k-tile-major columns
        Ew = wT.shape[1]
        return np.ascontiguousarray(
            wT.reshape(NK, HD, Ew).transpose(1, 0, 2).reshape(HD, NK * Ew))

    def scaled_bias(b_e):
        # column h holds head h's bias; heads 1-3 run the fp8 path whose
        # psum carries the 2**13 weight prescale on the data term
        bc = np.ascontiguousarray(b_e.reshape(NH, HD).T).astype(np.float32)
        bc[:, 1:] *= W8SCALE
        return bc

    xT = x_b.T  # [D, S]
    return {
        "xr": _bf16(xT.reshape(NK, HD, S).transpose(1, 0, 2)
                    .reshape(HD, NK * S)),
        "wqt": _f8(relayout(wq_e.T) * W8SCALE),
        "wkt": _f8(relayout(wk_e.T) * W8SCALE),
        "wqh": _bf16(relayout(np.ascontiguousarray(wq_e.T[:, 0:HD]))),
        "wkh": _bf16(relayout(np.ascontiguousarray(wk_e.T[:, 0:HD]))),
        "wvt": _bf16(relayout(wv_e.T)),
        "wot": _bf16(
            wo_e.T.reshape(NH, HD, D).transpose(1, 0, 2).reshape(HD, NH * D)),
        "bqc": scaled_bias(bq_e),
        "bkc": scaled_bias(bk_e),
        "bvc": np.ascontiguousarray(bv_e.reshape(NH, HD).T),
        "ones2d": _bf16(np.ones((HD, HD), np.float32)),
    }


# ---------------------------------------------------------------------------
# Entry point: full-input kernel with internal 8-way sharding
# ---------------------------------------------------------------------------

import os as _os

_NC_CACHE = {}


def _get_module():
    if "nc" not in _NC_CACHE:
        _NC_CACHE["nc"] = build_module(S=2048, D=2048, E=512)
    return _NC_CACHE["nc"]


def kernel(x, wq, bq, wk, bk, wv, bv, wo, bo):
    """Full inputs -> full output. 8 cores = 2 (batch) x 4 (head-group)."""
    from concourse import bass_utils

    x = np.asarray(x, dtype=np.float32)
    wq, bq = np.asarray(wq, np.float32), np.asarray(bq, np.float32)
    wk, bk = np.asarray(wk, np.float32), np.asarray(bk, np.float32)
    wv, bv = np.asarray(wv, np.float32), np.asarray(bv, np.float32)
    wo, bo = np.asarray(wo, np.float32), np.asarray(bo, np.float32)

    E = 512
    nc = _get_module()
    in_maps = []
    for c in range(8):
        b, g = divmod(c, 4)
        e = slice(g * E, (g + 1) * E)
        in_maps.append(make_in_map(
            x[b], wq[e], bq[e], wk[e], bk[e], wv[e], bv[e], wo[:, e]))

    trace = bool(int(_os.environ.get("ATTN_TRACE", "0")))
    kw = {}
    if trace:
        tmpdir = _os.environ.get("ATTN_TRACE_DIR") or None
        kw = dict(trace=True, tmpdir=tmpdir, trace_cores=[0])
    res = bass_utils.run_bass_kernel_spmd(
        nc, in_maps, core_ids=list(range(8)), **kw)
    if trace:
        print(f"HW exec time: {res.exec_time_ns} ns")
        _NC_CACHE["last_results"] = res

    y = np.empty((2, 2048, 2048), np.float32)
    for b in range(2):
        acc = np.asarray(res.results[4 * b]["out"], dtype=np.float32)
        for g in range(1, 4):
            acc += np.asarray(res.results[4 * b + g]["out"], dtype=np.float32)
        y[b] = acc + bo
    return y



# revision 21
# speedup vs baseline: 1.0171x; 1.0110x over previous
import sys
sys.path.insert(0, '/opt/trn_rl_repo')

"""Multi-head attention TP kernel for TRN2 — per-core program builder.

Sharding: 8 cores = 2 (batch) x 4 (head groups of 4 heads = 512 dims).
Each core computes, for its batch b and head-dim slice e:
    q = x[b] @ wq[e,:].T + bq[e]      (stored transposed: qT [E, S])
    k = x[b] @ wk[e,:].T + bk[e]      (kT [E, S])
    v = x[b] @ wv[e,:].T + bv[e]      (v [S, E])
    per head h (dh=128): ST = K-major score tiles, exp (no max-sub; scores
    bounded ~|3|), softmax denominator via DVE tensor_add accumulation of
    exp tiles + one all-ones cross-partition matmul, AV accumulated
    unnormalized, normalized on eviction.
    partial_out = attn_out @ wo[:, e].T   ([S, D]; host sums 8 partials + bo)

v2: one software-pipelined instruction stream, tuned from perfetto traces:
- The scalar-engine exp (~0.7-0.8 us per [128,512] tile, ~180 us/core
  total) hides under tensor-engine work: next-head K/Q projections and WO
  chunks are interleaved as "fillers" into the attention j-loops via a
  closure queue (2 pops/j, 5 during the last head when WO is available).
- AV matmuls are emitted 2 j-slots late (AV14/AV15 spill into the next
  block) so the last exp of a block never head-of-line-blocks the
  in-order PE queue; block finalize is deferred to the next block's j==1.
- Softmax denominator: DVE bf16 tensor_add chain over exp tiles + one
  all-ones cross-partition matmul; 1/denom via reciprocal_approx_fast
  (the exact DVE reciprocal costs 3.4 us/tile); V bias applied after
  normalization (softmax rows sum to 1), saving the bias matmuls.
- Q/K/V evictions on DVE, WO evictions on ACT (DVE congestion at the
  WO-heavy tail otherwise stalls the PE via the normalize chain).
- Startup: K/Q(h0) stream k-tile-wise under the x DMA (packed head-0
  weight columns arrive first; each dma_start costs ~0.74 us of serial
  Sync-engine trigger, so call count is minimized), with 3 V it-groups
  folded in; PSUM banks: 4 K + 1 Q + 3 V at startup, then 2 scores +
  2 AV-accum + 1 denom + 3 filler groups in steady state.
Measured ~363-366 us on 8 trn2 cores (vs 605 us for the 4-phase v1).
"""

import math
from collections import deque

import numpy as np

import concourse.bass as bass
import concourse.tile as tile
from concourse import bacc, mybir

F32 = mybir.dt.float32
BF16 = mybir.dt.bfloat16
AF = mybir.ActivationFunctionType


def build_module(
    S=2048,          # sequence per core (one batch)
    D=2048,          # model dim
    E=512,           # head dims per core (4 heads x 128)
    enable_asserts=False,
):
    HD = 128
    SC = 512
    NK = D // HD        # proj contraction tiles
    NH = E // HD        # heads per core
    NSC = S // SC       # s-chunks / i-blocks
    NJ = S // HD        # attention j tiles
    ND = D // SC        # WO n-chunks
    scale = 1.0 / math.sqrt(HD)

    # fp8 tensor-engine path for the steady-state K/Q projection fillers
    # (heads 1-3): DoubleRow perf mode processes two 128-row k-tiles per
    # matmul at 2x bf16 throughput. Weights are host-prescaled by 2**13 to
    # lift them out of e4m3's denormal range; the factor is divided back
    # out inside the exp activation's scale argument (biases are
    # host-prescaled to match), so descaling costs zero instructions.
    # Head 0 stays bf16: its K/Q run in the DMA-bound startup stream where
    # PE savings buy nothing, and keeping it unscaled lets bias column 0
    # serve the bf16 path unchanged.
    W8SCALE = 2.0 ** 13
    F8 = mybir.dt.float8e4

    nc = bacc.Bacc(
        "TRN2",
        target_bir_lowering=False,
        debug=False,
        enable_asserts=enable_asserts,
        num_devices=8,
    )

    xr = nc.dram_tensor("xr", [HD, NK * S], BF16, kind="ExternalInput").ap()
    wqt = nc.dram_tensor("wqt", [HD, NK * E], F8, kind="ExternalInput").ap()
    wkt = nc.dram_tensor("wkt", [HD, NK * E], F8, kind="ExternalInput").ap()
    # head-0 columns packed contiguously for a fast startup DMA
    wqh = nc.dram_tensor("wqh", [HD, NK * HD], BF16, kind="ExternalInput").ap()
    wkh = nc.dram_tensor("wkh", [HD, NK * HD], BF16, kind="ExternalInput").ap()
    wvt = nc.dram_tensor("wvt", [HD, NK * E], BF16, kind="ExternalInput").ap()
    wot = nc.dram_tensor("wot", [HD, NH * D], BF16, kind="ExternalInput").ap()
    bqc = nc.dram_tensor("bqc", [HD, NH], F32, kind="ExternalInput").ap()
    bkc = nc.dram_tensor("bkc", [HD, NH], F32, kind="ExternalInput").ap()
    bvc = nc.dram_tensor("bvc", [HD, NH], F32, kind="ExternalInput").ap()
    ones2d = nc.dram_tensor("ones2d", [HD, HD], BF16,
                            kind="ExternalInput").ap()
    out = nc.dram_tensor("out", [S, D], BF16, kind="ExternalOutput").ap()

    with tile.TileContext(nc) as tc:
        with (
            tc.tile_pool(name="main", bufs=1) as main,
            tc.tile_pool(name="consts", bufs=1) as consts,
        ):
            xall = main.tile([HD, NK, S], BF16)
            # fp8 copy of x, pair-contiguous for DoubleRow: the moving
            # operand [128, 2, SC] must be one linear 1024B run per
            # partition or the PE fetch path halves its rate
            x8 = main.tile([HD, NK // 2, NSC, 2, SC], F8)
            wq_sb = main.tile([HD, NK, E], F8)
            wk_sb = main.tile([HD, NK, E], F8)
            wqh_sb = main.tile([HD, NK, HD], BF16)
            wo_sb = main.tile([HD, NH, D], BF16)
            k_sb = main.tile([HD, 2, S], BF16)      # head ping-pong, kT
            v_sb = main.tile([HD, NJ, E], BF16)     # all heads
            q_sb = main.tile([HD, 2, SC], BF16)     # (h, ib) ping-pong
            outT = main.tile([HD, NH, S], BF16)

            bq_sb = consts.tile([HD, NH], F32)
            bk_sb = consts.tile([HD, NH], F32)
            bv_sb = consts.tile([HD, NH], F32)
            allones = consts.tile([HD, HD], BF16)

            # Startup-only tiles live in a scoped pool: closing it after the
            # startup phase releases their SBUF to the attention-phase pools
            # (the allocator tracks released-zone overlap dependencies).
            su_pool = tc.tile_pool(name="su", bufs=1)
            su = su_pool.__enter__()
            wkh_sb = su.tile([HD, NK, HD], BF16)
            wv_sb = su.tile([HD, NK, E], BF16)

            # Two hardware DGE queues: Sync streams x exclusively (the
            # startup k-stream consumes one x k-tile per ~1.7 us, so x must
            # never queue behind weight chunks), Scalar/Activation carries
            # every weight + const (idle until the first exp at ~95 us).
            # Each dma_start costs a ~0.6-0.7 us serial trigger on its
            # engine, so call count still matters per queue.
            NVS = 3   # V it-groups folded into the startup k-stream
            HK = NK // 2

            def dma_wv(g4):
                nc.sync.dma_start(
                    out=wv_sb[:, 4 * g4:4 * g4 + 4, :],
                    in_=wvt[:, 4 * g4 * E:(4 * g4 + 4) * E].rearrange(
                        "p (k e) -> p k e", e=E))

            # Sync queue: x with wv interleaved in exactly the order the
            # startup stream consumes (wv group g feeds V k-tiles from
            # x[4g+LAG]); single-queue delivery order tracks consumption.
            nc.sync.dma_start(out=xall[:, 0, 0:SC], in_=xr[:, 0:SC])
            nc.sync.dma_start(out=xall[:, 0, SC:], in_=xr[:, SC:S])
            nc.sync.dma_start(out=xall[:, 1, :], in_=xr[:, S:2 * S])
            for g in range(1, 8):
                nc.sync.dma_start(
                    out=xall[:, 2 * g:2 * g + 2, :],
                    in_=xr[:, 2 * g * S:(2 * g + 2) * S].rearrange(
                        "p (k s) -> p k s", s=S))
                if g in (1, 3, 5):
                    dma_wv(g // 2)
            dma_wv(3)

            # Scalar queue: startup weights first (wkh/wqh k0-half feeds the
            # first matmul), biases before the first K eviction,
            # steady-state weights last.
            nc.scalar.dma_start(
                out=wkh_sb[:, 0:HK, :],
                in_=wkh[:, :HK * HD].rearrange("p (k e) -> p k e", e=HD))
            nc.scalar.dma_start(
                out=wqh_sb[:, 0:HK, :],
                in_=wqh[:, :HK * HD].rearrange("p (k e) -> p k e", e=HD))
            nc.scalar.dma_start(
                out=wkh_sb[:, HK:, :],
                in_=wkh[:, HK * HD:].rearrange("p (k e) -> p k e", e=HD))
            nc.scalar.dma_start(
                out=wqh_sb[:, HK:, :],
                in_=wqh[:, HK * HD:].rearrange("p (k e) -> p k e", e=HD))
            nc.scalar.dma_start(out=bk_sb, in_=bkc)
            nc.scalar.dma_start(out=bq_sb, in_=bqc)
            nc.scalar.dma_start(out=bv_sb, in_=bvc)
            nc.scalar.dma_start(out=allones, in_=ones2d)
            for g4 in range(NK // 4):
                nc.scalar.dma_start(
                    out=wk_sb[:, 4 * g4:4 * g4 + 4, :],
                    in_=wkt[:, 4 * g4 * E:(4 * g4 + 4) * E].rearrange(
                        "p (k e) -> p k e", e=E))
            for g4 in range(NK // 4):
                nc.scalar.dma_start(
                    out=wq_sb[:, 4 * g4:4 * g4 + 4, :],
                    in_=wqt[:, 4 * g4 * E:(4 * g4 + 4) * E].rearrange(
                        "p (k e) -> p k e", e=E))
            nc.scalar.dma_start(
                out=wo_sb, in_=wot.rearrange("p (k d) -> p k d", d=D))

            # x -> fp8 casts, k-tile by k-tile as the x DMAs land. Mostly on
            # DVE (gpsimd CAST is ~1.9us per [128,512] tile vs ~0.7 on DVE);
            # both engines are idle during the startup stream. Consumed by
            # the h1-3 K/Q fillers from ~95us on.
            for kk in range(NK):
                for c in range(NSC):
                    eng = nc.gpsimd if (kk * NSC + c) % 4 == 3 else nc.vector
                    eng.tensor_copy(
                        x8[:, kk // 2, c, kk % 2, :],
                        xall[:, kk, c * SC:(c + 1) * SC])

            # ---- startup: K(h0), Q(h0, ib0), V it 0..NVS-1 k-tile-wise,
            # ---- then the remaining V it-groups.
            with tc.tile_pool(name="psStart", bufs=1, space="PSUM") as psS0:
                psK0 = [psS0.tile([HD, SC], F32, tag=f"k{si}",
                                  name=f"psk{si}") for si in range(NSC)]
                psQ0 = psS0.tile([HD, SC], F32, tag="q", name="psq0")
                psV0 = [psS0.tile([HD, E], F32, tag=f"v{it}",
                                  name=f"psv{it}") for it in range(NVS)]

                # PE warmup: the tensor engine clock ramps over ~3 us of
                # continuous work. Burn dummy matmuls on a memset scratch
                # tile during the otherwise-idle DMA-latency window so the
                # first real matmuls run at full clock.
                N_WARM, WARM_COLS = 20, 256
                warm = su.tile([HD, WARM_COLS], BF16)
                nc.gpsimd.memset(warm, 0)
                for _ in range(N_WARM):
                    nc.tensor.matmul(psK0[0][:, 0:WARM_COLS],
                                     warm[:, 0:HD], warm,
                                     start=True, stop=True)
                # V trails K/Q by LAG k-tiles so its wv chunks (DMA'd
                # behind x) always arrive before the in-order PE stream
                # reaches them
                LAG = 4
                for kk in range(NK + LAG):
                    if kk < NK:
                        st, sp = kk == 0, kk == NK - 1
                        for si in range(NSC):
                            nc.tensor.matmul(
                                psK0[si], wkh_sb[:, kk, :],
                                xall[:, kk, si * SC:(si + 1) * SC],
                                start=st, stop=sp)
                        nc.tensor.matmul(
                            psQ0, wqh_sb[:, kk, :], xall[:, kk, 0:SC],
                            start=st, stop=sp)
                    vk = kk - LAG
                    if 0 <= vk < NK:
                        for it in range(NVS):
                            nc.tensor.matmul(
                                psV0[it],
                                xall[:, vk, it * HD:(it + 1) * HD],
                                wv_sb[:, vk, :], start=(vk == 0),
                                stop=(vk == NK - 1))
                for si in range(NSC):
                    nc.vector.tensor_scalar_add(
                        k_sb[:, 0, si * SC:(si + 1) * SC], psK0[si],
                        bk_sb[:, 0:1])
                nc.vector.tensor_scalar_add(q_sb[:, 0, :], psQ0, bq_sb[:, 0:1])
                for it in range(NVS):
                    nc.vector.tensor_copy(v_sb[:, it, :], psV0[it])

                for it in range(NVS, NJ):
                    psV = psS0.tile([HD, E], F32, tag=f"v{it % NVS}",
                                    name="psv")
                    for kk in range(NK):
                        nc.tensor.matmul(
                            psV, xall[:, kk, it * HD:(it + 1) * HD],
                            wv_sb[:, kk, :], start=(kk == 0),
                            stop=(kk == NK - 1))
                    nc.vector.tensor_copy(v_sb[:, it, :], psV)

            su_pool.__exit__(None, None, None)

            # ---------------- pipelined attention + fillers ----------------
            with (
                tc.tile_pool(name="psS", bufs=2, space="PSUM") as psS_pool,
                tc.tile_pool(name="psO", bufs=2, space="PSUM") as psO_pool,
                tc.tile_pool(name="psBC", bufs=1, space="PSUM") as psBC_pool,
                tc.tile_pool(name="psF", bufs=3, space="PSUM") as psF_pool,
                tc.tile_pool(name="es", bufs=6) as es_pool,
                tc.tile_pool(name="accp", bufs=3) as acc_pool,
                tc.tile_pool(name="recipp", bufs=2) as recip_pool,
                tc.tile_pool(name="og", bufs=2) as og_pool,
            ):
                filler = deque()

                def emit_filler(n):
                    for _ in range(n):
                        if filler:
                            filler.popleft()()

                DR = mybir.MatmulPerfMode.DoubleRow
                NP = NK // 2    # DoubleRow k-tile pairs

                def enq_q(h, ib, qb):
                    psQ = psF_pool.tile([HD, SC], F32, tag="f", name="psq")
                    if h == 0:
                        # bf16 path (unscaled bias column 0)
                        def op(kk, psQ=psQ, ib=ib, qb=qb):
                            nc.tensor.matmul(
                                psQ, wqh_sb[:, kk, :],
                                xall[:, kk, ib * SC:(ib + 1) * SC],
                                start=(kk == 0), stop=(kk == NK - 1))
                            if kk == NK - 1:
                                nc.vector.tensor_scalar_add(
                                    q_sb[:, qb, :], psQ, bq_sb[:, 0:1])
                        for kk in range(NK):
                            filler.append(lambda kk=kk, op=op: op(kk))
                        return

                    def op(t, psQ=psQ, h=h, ib=ib, qb=qb):
                        nc.tensor.matmul(
                            psQ, wq_sb[:, 2 * t:2 * t + 2, h * HD:(h + 1) * HD],
                            x8[:, t, ib, :, :],
                            start=(t == 0), stop=(t == NP - 1), perf_mode=DR)
                        if t == NP - 1:
                            nc.vector.tensor_scalar_add(
                                q_sb[:, qb, :], psQ, bq_sb[:, h:h + 1])
                    for t in range(NP):
                        filler.append(lambda t=t, op=op: op(t))

                def enq_k(h, si, kb):
                    psK = psF_pool.tile([HD, SC], F32, tag="f", name="psk")

                    def op(t, psK=psK, h=h, si=si, kb=kb):
                        nc.tensor.matmul(
                            psK, wk_sb[:, 2 * t:2 * t + 2, h * HD:(h + 1) * HD],
                            x8[:, t, si, :, :],
                            start=(t == 0), stop=(t == NP - 1), perf_mode=DR)
                        if t == NP - 1:
                            nc.vector.tensor_scalar_add(
                                k_sb[:, kb, si * SC:(si + 1) * SC], psK,
                                bk_sb[:, h:h + 1])
                    for t in range(NP):
                        filler.append(lambda t=t, op=op: op(t))

                def enq_wo(ibp):
                    for it4 in range(4):
                        it = ibp * 4 + it4
                        og = og_pool.tile([HD, D], BF16, tag="og", name="og")
                        for nn in range(ND):
                            psW = psF_pool.tile([HD, SC], F32, tag="f",
                                                name="psw")

                            def op(kkh, psW=psW, og=og, it=it, nn=nn, ibp=ibp):
                                nc.tensor.matmul(
                                    psW,
                                    outT[:, kkh, it * HD:(it + 1) * HD],
                                    wo_sb[:, kkh, nn * SC:(nn + 1) * SC],
                                    start=(kkh == 0), stop=(kkh == NH - 1))
                                if kkh == NH - 1:
                                    if ibp == 3:
                                        # drain: both ACT and DVE are idle;
                                        # alternate so the eviction chain
                                        # keeps pace with the WO matmuls
                                        if (it * ND + nn) % 2:
                                            nc.scalar.copy(
                                                og[:, nn * SC:(nn + 1) * SC],
                                                psW)
                                        else:
                                            nc.vector.tensor_copy(
                                                og[:, nn * SC:(nn + 1) * SC],
                                                psW)
                                    elif ibp == 2:
                                        # last attention block: DVE (ACT
                                        # paces the exp chain there)
                                        nc.vector.tensor_copy(
                                            og[:, nn * SC:(nn + 1) * SC], psW)
                                    else:
                                        nc.scalar.copy(
                                            og[:, nn * SC:(nn + 1) * SC], psW)
                                    if it == NJ - 1:
                                        # tail: per-chunk DMA right after
                                        # each eviction so the final
                                        # transfer overlaps the drain
                                        nc.sync.dma_start(
                                            out=out[it * HD:(it + 1) * HD,
                                                    nn * SC:(nn + 1) * SC],
                                            in_=og[:, nn * SC:(nn + 1) * SC])
                                    elif nn == ND - 1:
                                        nc.sync.dma_start(
                                            out=out[it * HD:(it + 1) * HD, :],
                                            in_=og)
                            for kkh in range(NH):
                                filler.append(lambda kkh=kkh, op=op: op(kkh))

                pending_fin = [None]
                # AV matmuls are emitted 2 j-slots late: the last exp of a
                # block lags the PE by ~1.4 us, and an eagerly-emitted AV15
                # head-of-line-blocks the in-order PE queue at every block
                # boundary. AV14/AV15 spill into the next block's stream.
                av_q = deque()

                for h in range(NH):
                    kb = h % 2
                    for ib in range(NSC):
                        i0 = ib * SC
                        qb = (h * NSC + ib) % 2
                        if ib < NSC - 1:
                            enq_q(h, ib + 1, qb ^ 1)
                        elif h < NH - 1:
                            enq_q(h + 1, 0, qb ^ 1)
                        if h < NH - 1:
                            enq_k(h + 1, ib, kb ^ 1)

                        if h == NH - 1:
                            # leave ~16 WO fillers for the epilogue: the
                            # deferred AV14/AV15 flush needs covering work
                            # or it stalls on the final block's last exp
                            npop = 4 if ib == NSC - 1 else 5
                        else:
                            npop = 2
                        psO = psO_pool.tile([HD, SC], F32, tag="o", name="pso")
                        es_t = [None] * NJ
                        acc = None

                        def emit_av(j, h=h, psO=psO, es_t=es_t):
                            nc.tensor.matmul(
                                psO, v_sb[:, j, h * HD:(h + 1) * HD],
                                es_t[j],
                                start=(j == 0), stop=(j == NJ - 1))

                        for j in range(NJ):
                            psS = psS_pool.tile([HD, SC], F32, tag="s",
                                                name="pss")
                            nc.tensor.matmul(
                                psS, k_sb[:, kb, j * HD:(j + 1) * HD],
                                q_sb[:, qb, :], start=True, stop=True)
                            emit_filler(npop)
                            if len(av_q) >= 2:
                                av_q.popleft()()
                            if j == 1:
                                # after the AV15 pop above — fin's normalize
                                # reads psO and must be emitted after it
                                if pending_fin[0] is not None:
                                    pending_fin[0]()
                                    pending_fin[0] = None
                                if h == NH - 1 and ib >= 1:
                                    enq_wo(ib - 1)
                            es = es_pool.tile([HD, SC], BF16, tag="es",
                                              name="es")
                            # h>=1 scores carry the 2**13 weight prescale on
                            # both q and k; divide it back out here for free
                            sc_h = scale if h == 0 else scale / (W8SCALE ** 2)
                            nc.scalar.activation(es, psS, AF.Exp, scale=sc_h)
                            es_t[j] = es
                            av_q.append(lambda j=j, f=emit_av: f(j))
                            if j >= 1:
                                nacc = acc_pool.tile([HD, SC], BF16, tag="acc",
                                                     name="acc")
                                if j == 1:
                                    nc.vector.tensor_add(nacc, es_t[0], es)
                                else:
                                    nc.vector.tensor_add(nacc, acc, es)
                                acc = nacc

                        def fin(h=h, i0=i0, psO=psO, acc=acc):
                            psBC = psBC_pool.tile([HD, SC], F32, tag="bc",
                                                  name="psbc")
                            nc.tensor.matmul(psBC, allones, acc,
                                             start=True, stop=True)
                            recip = recip_pool.tile([HD, SC], F32, tag="r",
                                                    name="recip")
                            nc.vector.reciprocal_approx_fast(
                                out=recip, in_=psBC)
                            nc.vector.tensor_mul(
                                outT[:, h, i0:i0 + SC], psO, recip)
                            nc.vector.tensor_scalar_add(
                                outT[:, h, i0:i0 + SC],
                                outT[:, h, i0:i0 + SC], bv_sb[:, h:h + 1])
                        pending_fin[0] = fin

                # epilogue: cover the exp15/normalize latency of the last
                # block with leftover WO fillers before flushing the
                # deferred AVs and the final WO chunk
                emit_filler(min(len(filler), 8))
                while av_q:
                    av_q.popleft()()
                pending_fin[0]()
                pending_fin[0] = None
                emit_filler(min(len(filler), 12))
                enq_wo(NSC - 1)
                emit_filler(len(filler))

    nc.compile()
    return nc


# ---------------------------------------------------------------------------
# Host-side sharding helpers
# ---------------------------------------------------------------------------

def _bf16(a):
    import ml_dtypes
    return np.asarray(a).astype(ml_dtypes.bfloat16)


def _f8(a):
    import ml_dtypes
    return np.asarray(a).astype(ml_dtypes.float8_e4m3)


def make_in_map(x_b, wq_e, bq_e, wk_e, bk_e, wv_e, bv_e, wo_e):
    """Per-core input dict. x_b [S, D]; w*_e [E, D] row slices; wo_e [D, E]
    column slice; b*_e [E]."""
    E = wq_e.shape[0]
    S, D = x_b.shape
    HD = 128
    NH = E // HD
    NK = D // HD

    W8SCALE = np.float32(2.0 ** 13)

    def relayout(wT):  # [D, E'] -> [HD, NK*E'] with k-tile-major columns
        Ew = wT.shape[1]
        return np.ascontiguousarray(
            wT.reshape(NK, HD, Ew).transpose(1, 0, 2).reshape(HD, NK * Ew))

    def scaled_bias(b_e):
        # column h holds head h's bias; heads 1-3 run the fp8 path whose
        # psum carries the 2**13 weight prescale on the data term
        bc = np.ascontiguousarray(b_e.reshape(NH, HD).T).astype(np.float32)
        bc[:, 1:] *= W8SCALE
        return bc

    xT = x_b.T  # [D, S]
    return {
        "xr": _bf16(xT.reshape(NK, HD, S).transpose(1, 0, 2)
                    .reshape(HD, NK * S)),
        "wqt": _f8(relayout(wq_e.T) * W8SCALE),
        "wkt": _f8(relayout(wk_e.T) * W8SCALE),
        "wqh": _bf16(relayout(np.ascontiguousarray(wq_e.T[:, 0:HD]))),
        "wkh": _bf16(relayout(np.ascontiguousarray(wk_e.T[:, 0:HD]))),
        "wvt": _bf16(relayout(wv_e.T)),
        "wot": _bf16(
            wo_e.T.reshape(NH, HD, D).transpose(1, 0, 2).reshape(HD, NH * D)),
        "bqc": scaled_bias(bq_e),
        "bkc": scaled_bias(bk_e),
        "bvc": np.ascontiguousarray(bv_e.reshape(NH, HD).T),
        "ones2d": _bf16(np.ones((HD, HD), np.float32)),
    }


# ---------------------------------------------------------------------------
# Entry point: full-input kernel with internal 8-way sharding
# ---------------------------------------------------------------------------

import os as _os

_NC_CACHE = {}


def _get_module():
    if "nc" not in _NC_CACHE:
        _NC_CACHE["nc"] = build_module(S=2048, D=2048, E=512)
    return _NC_CACHE["nc"]


def kernel(x, wq, bq, wk, bk, wv, bv, wo, bo):
    """Full inputs -> full output. 8 cores = 2 (batch) x 4 (head-group)."""
    from concourse import bass_utils

    x = np.asarray(x, dtype=np.float32)
    wq, bq = np.asarray(wq, np.float32), np.asarray(bq, np.float32)
    wk, bk = np.asarray(wk, np.float32), np.asarray(bk, np.float32)
    wv, bv = np.asarray(wv, np.float32), np.asarray(bv, np.float32)
    wo, bo = np.asarray(wo, np.float32), np.asarray(bo, np.float32)

    E = 512
    nc = _get_module()
    in_maps = []
    for c in range(8):
        b, g = divmod(c, 4)
        e = slice(g * E, (g + 1) * E)
        in_maps.append(make_in_map(
            x[b], wq[e], bq[e], wk[e], bk[e], wv[e], bv[e], wo[:, e]))

    trace = bool(int(_os.environ.get("ATTN_TRACE", "0")))
    kw = {}
    if trace:
        tmpdir = _os.environ.get("ATTN_TRACE_DIR") or None
        kw = dict(trace=True, tmpdir=tmpdir, trace_cores=[0])
    res = bass_utils.run_bass_kernel_spmd(
        nc, in_maps, core_ids=list(range(8)), **kw)
    if trace:
        print(f"HW exec time: {res.exec_time_ns} ns")
        _NC_CACHE["last_results"] = res

    y = np.empty((2, 2048, 2048), np.float32)
    for b in range(2):
        acc = np.asarray(res.results[4 * b]["out"], dtype=np.float32)
        for g in range(1, 4):
            acc += np.asarray(res.results[4 * b + g]["out"], dtype=np.float32)
        y[b] = acc + bo
    return y

